# revision 2
# baseline (speedup 1.0000x reference)
"""Trainium2 Bass kernel: causal multi-head group attention (GQA) with RoPE.

v2: bf16 datapath + software-pipelined phase interleaving + engine rebalance.

Full-input contract: kernel(**inputs) takes the unsharded inputs and returns
the full output. Internally shards across 8 NeuronCores:
  core c -> (batch b = c // 4, head-group g = c % 4)
Each core computes 4 q heads + their single kv group end-to-end. The host
unshard step sums the 4 row-parallel out-proj partials and adds the bias.

Design notes (cost-model driven):
 - All matmul operands bf16 (1 cycle/row at any output width); PSUM f32.
 - Emission interleaves proj(sp) / attention(sp-1) / out-proj(sp-2) so the
   tensor engine (the pacing engine, ~205us of work) never idles while the
   Act engine grinds through exp (the attention pacer).
 - Engine assignment: exp -> Act; rope / rs-accum / mask / reciprocal -> DVE
   (bf16 SBUF-only ops hit the 4x DVE mode); softmax y-scale and out-proj
   PSUM->SBUF copies -> Pool (otherwise idle); bias add -> host.
 - V is projected with x as the stationary operand, yielding [s, d] tiles
   directly (no PE transposes, single PSUM->SBUF copy per pass).
"""

import os
import sys
from contextlib import ExitStack, nullcontext
from math import sqrt

for _p in ("/opt/trn_rl_repo", "/root/.axon_site/_ro/trn_rl_repo"):
    if os.path.isdir(_p) and _p not in sys.path:
        sys.path.insert(0, _p)

import numpy as np
import concourse.bacc as bacc
import concourse.tile as tile
import concourse.mybir as mybir
from concourse.bass_utils import run_bass_kernel_spmd

F32 = mybir.dt.float32
BF16 = mybir.dt.bfloat16
EXP = mybir.ActivationFunctionType.Exp
NPBF16 = mybir.dt.np(BF16)

N_CORES = 8
TP = 4            # head-group parallel degree (within one batch element)
BATCH = 2
D = 128           # head dim
NHL = 4           # q heads per core
ROPE_BASE = 10000.0

S_FULL = 2048     # context length
E_FULL = 2048     # model dim


def build_program(S, E, QC=512, PW=512, n_cores=N_CORES, reps=1, debug=False):
    EC = E // 128     # contraction chunks over model dim
    NSP = S // PW     # passes over the sequence (also q-chunks: QC == PW)
    assert QC == PW == 512
    scale = 1.0 / sqrt(D)

    nc = bacc.Bacc("TRN2", target_bir_lowering=False, debug=False,
                   num_devices=n_cores)

    xT = nc.dram_tensor("xT", [E, S], BF16, kind="ExternalInput").ap()
    Wq = nc.dram_tensor("Wq", [E, NHL * D], BF16, kind="ExternalInput").ap()
    Wk = nc.dram_tensor("Wk", [E, D], BF16, kind="ExternalInput").ap()
    Wv = nc.dram_tensor("Wv", [E, D], BF16, kind="ExternalInput").ap()
    Wo = nc.dram_tensor("Wo", [NHL * D, E], BF16, kind="ExternalInput").ap()
    sinT = nc.dram_tensor("sinT", [D, S], BF16, kind="ExternalInput").ap()
    sinTs = nc.dram_tensor("sinTs", [D, S], BF16, kind="ExternalInput").ap()
    mask_in = nc.dram_tensor("mask_in", [128, 128], BF16, kind="ExternalInput").ap()
    ones_col = nc.dram_tensor("ones_col", [128, 1], BF16, kind="ExternalInput").ap()
    ones_row = nc.dram_tensor("ones_row", [1, 128], BF16, kind="ExternalInput").ap()
    out = nc.dram_tensor("out", [S, E], BF16, kind="ExternalOutput").ap()
    if debug:
        dbg_q = [nc.dram_tensor(f"dbg_q{h}", [128, S], BF16,
                                kind="ExternalOutput").ap() for h in range(NHL)]
        dbg_k = nc.dram_tensor("dbg_k", [128, S], BF16,
                               kind="ExternalOutput").ap()
        dbg_v = nc.dram_tensor("dbg_v", [128, S], BF16,
                               kind="ExternalOutput").ap()
        dbg_y = [nc.dram_tensor(f"dbg_y{h}", [128, S], BF16,
                                kind="ExternalOutput").ap() for h in range(NHL)]

    with tile.TileContext(nc) as tc, \
         (tc.For_i(0, reps, 1) if reps > 1 else nullcontext()), \
         ExitStack() as top:
        pers = top.enter_context(tc.tile_pool(name="pers", bufs=1))
        qkT = [pers.tile([128, S], BF16, tag=f"qkT{g}", name=f"qkT{g}")
               for g in range(NHL + 1)]      # 4 q heads + k, [d, s] layout
        vA = pers.tile([128, S], BF16, name="vA")      # [:,128k:128k+128]=[s,d]
        yT = [pers.tile([128, S], BF16, tag=f"yT{h}", name=f"yT{h}")
              for h in range(NHL)]
        wq = pers.tile([128, EC * NHL * D], BF16, tag="wq", name="wq")
        wk = pers.tile([128, EC * D], BF16, tag="wk", name="wk")
        wv = pers.tile([128, EC * D], BF16, tag="wv", name="wv")
        wo = pers.tile([128, NHL * E], BF16, tag="wo", name="wo")
        sin_sb = pers.tile([128, S], BF16, tag="sin", name="sin_sb")
        sins_sb = pers.tile([128, S], BF16, tag="sins", name="sins_sb")
        mask_sb = pers.tile([128, 128], BF16, tag="mask", name="mask_sb")
        onesc = pers.tile([128, 1], BF16, tag="onesc", name="onesc")
        onesr = pers.tile([1, 128], BF16, tag="onesr", name="onesr")

        xpool = top.enter_context(tc.tile_pool(name="xt", bufs=2))
        opool = top.enter_context(tc.tile_pool(name="osb", bufs=2))
        hot = top.enter_context(tc.tile_pool(name="hot", bufs=1))
        psum = top.enter_context(tc.tile_pool(name="psum", bufs=1, space="PSUM"))

        xT_v = xT.rearrange("(a p) m -> p a m", p=128)
        wq_v3 = wq.rearrange("p (a n) -> p a n", a=EC)
        Wq_v3 = Wq.rearrange("(a p) n -> p a n", p=128)
        wo_v3 = wo.rearrange("p (a n) -> p a n", a=NHL)
        Wo_v3 = Wo.rearrange("(a p) n -> p a n", p=128)

        xt = [None] * NSP  # per-pass x tiles

        def load_x(sp, e0, e1):
            """One DMA loading x chunks [e0, e1) for pass sp."""
            if xt[sp] is None:
                xt[sp] = xpool.tile([128, EC * PW], BF16, tag="xt",
                                    name=f"xt{sp}")
            v3 = xt[sp].rearrange("p (a m) -> p a m", a=EC)
            nc.sync.dma_start(v3[:, e0:e1, :],
                              xT_v[:, e0:e1, PW * sp:PW * (sp + 1)])

        def emit_xdma(sp):
            return [(60, lambda sp=sp: load_x(sp, 0, EC))]

        # ------------- stream builders (lists of (pe_ns, thunk)) ----------
        def proj_stream(sp):
            units = []
            lo, hi = PW * sp, PW * (sp + 1)

            def xsl(e):
                return xt[sp][:, PW * e:PW * (e + 1)]

            def qk_stat(g, e):
                if g < NHL:
                    return wq[:, NHL * D * e + D * g:NHL * D * e + D * (g + 1)]
                return wk[:, D * e:D * (e + 1)]

            def rope_of(g, ps, lo=lo, hi=hi, sp=sp):
                full = qkT[g]
                t = full[:, lo:hi]
                nc.vector.tensor_copy(t, ps[:])
                tmp = hot.tile([128, PW], BF16, tag="ropetmp", bufs=2,
                               name=f"rt{sp}_{g}")
                nc.sync.dma_start(tmp[0:64, :], full[64:128, lo:hi])
                nc.sync.dma_start(tmp[64:128, :], full[0:64, lo:hi])
                nc.vector.tensor_mul(tmp[:], tmp[:], sins_sb[:, lo:hi])
                nc.vector.tensor_mul(t, t, sin_sb[:, lo:hi])
                nc.vector.tensor_add(t, t, tmp[:])

            if sp == 0:
                # Pass 0 runs e-major across all 5 q/k groups (one PSUM bank
                # each, via distinct tags) so the PE consumes x chunks as the
                # startup DMAs deliver them instead of stalling group-major.
                tags = ["proj", "proj", "st", "st", "yps"]
                ps0 = [psum.tile([128, PW], F32, tag=tags[g], bufs=2,
                                 name=f"pj0_{g}") for g in range(NHL + 1)]

                def emm(e):
                    for g in range(NHL + 1):
                        nc.tensor.matmul(ps0[g][:], qk_stat(g, e), xsl(e),
                                         start=(e == 0), stop=(e == EC - 1))
                for e in range(EC):
                    units.append((1065, lambda e=e: emm(e)))
                for g in range(NHL + 1):
                    units.append((100, lambda g=g: rope_of(g, ps0[g])))
            else:
                # 4 q heads + k: groups of 16 accumulating matmuls -> copy
                for g in range(NHL + 1):
                    ps_box = []

                    def mm4(e0, g=g, ps_box=ps_box, sp=sp):
                        if not ps_box:
                            ps_box.append(psum.tile([128, PW], F32, tag="proj",
                                                    bufs=2, name=f"pj{sp}_{g}"))
                        ps = ps_box[0]
                        for e in range(e0, e0 + 4):
                            nc.tensor.matmul(ps[:], qk_stat(g, e), xsl(e),
                                             start=(e == 0), stop=(e == EC - 1))
                    for e0 in range(0, EC, 4):
                        units.append((854, lambda e0=e0, f=mm4: f(e0)))
                    units.append((100, lambda g=g, ps_box=ps_box:
                                  rope_of(g, ps_box[0])))

            # v: [s, d] tiles via x-stationary matmuls. One accumulation
            # group per 128-row s-chunk, each in its own PSUM tile: a matmul
            # with start=True zeroes its whole PSUM bank, so interleaved
            # groups must not share one.
            def vgroup(j):
                vt = psum.tile([128, D], F32, tag="proj", bufs=2,
                               name=f"vps{sp}_{j}")
                for e in range(EC):
                    nc.tensor.matmul(
                        vt[:],
                        xt[sp][:, PW * e + 128 * j:PW * e + 128 * (j + 1)],
                        wv[:, D * e:D * (e + 1)],
                        start=(e == 0), stop=(e == EC - 1))
                nc.vector.tensor_copy(
                    vA[:, lo + 128 * j:lo + 128 * (j + 1)], vt[:])
            for j in range(4):
                units.append((900, lambda j=j: vgroup(j)))
            return units

        def attn_stream(sp):
            units = []
            nki = 4 * (sp + 1)
            for h in range(NHL):
                yps_box, rs_box = [], []

                def tilework(ki, h=h, yps_box=yps_box, rs_box=rs_box, sp=sp,
                             nki=nki):
                    if not yps_box:
                        yps_box.append(psum.tile([128, QC], F32, tag="yps",
                                                 bufs=2, name=f"yps{h}_{sp}"))
                        rs_box.append(hot.tile([128, QC], BF16, tag="rs",
                                               bufs=2, name=f"rs{h}_{sp}"))
                    yps, rs = yps_box[0], rs_box[0]
                    off = 128 * ki - QC * sp
                    qlo = max(0, off)
                    st = psum.tile([128, QC], F32, tag="st", bufs=2,
                                   name=f"st{h}_{sp}_{ki}")
                    nc.tensor.matmul(
                        st[:, qlo:QC], qkT[NHL][:, 128 * ki:128 * (ki + 1)],
                        qkT[h][:, QC * sp + qlo:QC * (sp + 1)],
                        start=True, stop=True)
                    if ki == 0:
                        dst = rs
                    else:
                        dst = hot.tile([128, QC], BF16, tag="pt", bufs=4,
                                       name=f"pt{h}_{sp}_{ki}")
                    nc.scalar.activation(dst[:, qlo:QC], st[:, qlo:QC], EXP,
                                         scale=scale)
                    if off >= 0:
                        nc.gpsimd.tensor_mul(dst[:, qlo:qlo + 128],
                                             dst[:, qlo:qlo + 128], mask_sb[:])
                    if ki != 0:
                        nc.vector.tensor_add(rs[:, qlo:QC], rs[:, qlo:QC],
                                             dst[:, qlo:QC])
                    nc.tensor.matmul(yps[:, qlo:QC],
                                     vA[:, 128 * ki:128 * (ki + 1)],
                                     dst[:, qlo:QC],
                                     start=(ki == 0), stop=(ki == nki - 1))
                for ki in range(nki):
                    units.append((430, lambda ki=ki, f=tilework: f(ki)))

                def fin(h=h, yps_box=yps_box, rs_box=rs_box, sp=sp):
                    rsum = psum.tile([1, QC], F32, tag="aux", bufs=2,
                                     name=f"rsum{h}_{sp}")
                    nc.tensor.matmul(rsum[:], onesc[:], rs_box[0][:],
                                     start=True, stop=True)
                    rinv = hot.tile([1, QC], BF16, tag="rinv", bufs=2,
                                    name=f"rinv{h}_{sp}")
                    with nc.allow_low_precision(reason="bf16 softmax denom"):
                        nc.vector.reciprocal(rinv[:], rsum[:])
                    rb = psum.tile([128, QC], F32, tag="aux", bufs=2,
                                   name=f"rb{h}_{sp}")
                    nc.tensor.matmul(rb[:], onesr[:], rinv[:],
                                     start=True, stop=True)
                    rb_sb = hot.tile([128, QC], F32, tag="rbs", bufs=2,
                                     name=f"rbs{h}_{sp}")
                    nc.scalar.copy(rb_sb[:], rb[:])
                    with nc.allow_low_precision(reason="bf16 y"):
                        nc.vector.tensor_mul(yT[h][:, QC * sp:QC * (sp + 1)],
                                             yps_box[0][:], rb_sb[:])
                units.append((430, fin))
            return units

        def outproj_stream(sp):
            units = []
            for si in range(4 * sp, 4 * (sp + 1)):
                osb_box = []

                def njwork(nj, si=si, osb_box=osb_box, sp=sp):
                    if not osb_box:
                        osb_box.append(opool.tile([128, E], BF16, tag="osb",
                                                  name=f"osb{si}"))
                    ops = psum.tile([128, 512], F32, tag="aux", bufs=2,
                                    name=f"ops{si}_{nj}")
                    for h in range(NHL):
                        nc.tensor.matmul(
                            ops[:], yT[h][:, 128 * si:128 * (si + 1)],
                            wo[:, E * h + 512 * nj:E * h + 512 * (nj + 1)],
                            start=(h == 0), stop=(h == NHL - 1))
                    dst = osb_box[0][:, 512 * nj:512 * (nj + 1)]
                    # Pool can't read PSUM; Act has slack in early passes
                    # (light attention), DVE in late ones (no projections)
                    if sp < 2:
                        nc.scalar.copy(dst, ops[:])
                    else:
                        nc.vector.tensor_copy(dst, ops[:])
                for nj in range(E // 512):
                    units.append((852, lambda nj=nj, f=njwork: f(nj)))

                def outdma(si=si, osb_box=osb_box, sp=sp):
                    rows = out[128 * si:128 * (si + 1), :]
                    if sp == NSP - 1 and si == 4 * (sp + 1) - 1:
                        # final chunk: 4 smaller DMAs so the tail drains early
                        for nj in range(4):
                            nc.sync.dma_start(
                                rows[:, 512 * nj:512 * (nj + 1)],
                                osb_box[0][:, 512 * nj:512 * (nj + 1)])
                    else:
                        nc.sync.dma_start(rows, osb_box[0][:])
                units.append((60, outdma))
            return units

        def merge(streams):
            streams = [s for s in streams if s]
            totals = [sum(w for w, _ in s) for s in streams]
            prog = [0.0] * len(streams)
            idx = [0] * len(streams)
            while True:
                cands = [i for i in range(len(streams))
                         if idx[i] < len(streams[i])]
                if not cands:
                    break
                i = min(cands, key=lambda i: prog[i] / totals[i])
                w, t = streams[i][idx[i]]
                idx[i] += 1
                prog[i] += w
                t()

        # -------------------- preload + pipeline schedule -----------------
        # Startup order tuned for serialized HWDGE descriptor generation:
        # block-interleave wq / wk / x0 so pass-0's e-major projection can
        # consume chunks as they land; tables/constants follow (their
        # consumers tolerate the latency).
        wk_v3 = wk.rearrange("p (a n) -> p a n", a=EC)
        Wk_v3 = Wk.rearrange("(a p) n -> p a n", p=128)
        for b0, b1 in ((0, 2), (2, 4), (4, 8), (8, 12), (12, 16)):
            nc.sync.dma_start(wq_v3[:, b0:b1, :], Wq_v3[:, b0:b1, :])
            nc.sync.dma_start(wk_v3[:, b0:b1, :], Wk_v3[:, b0:b1, :])
            load_x(0, b0, b1)
        nc.sync.dma_start(wv.rearrange("p (a n) -> p a n", a=EC),
                          Wv.rearrange("(a p) n -> p a n", p=128))
        nc.sync.dma_start(sin_sb[:], sinT[:])
        nc.sync.dma_start(sins_sb[:], sinTs[:])
        nc.sync.dma_start(mask_sb[:], mask_in[:])
        nc.sync.dma_start(onesc[:], ones_col[:])
        nc.sync.dma_start(onesr[:], ones_row[:])

        def insert_unit(units, frac, thunk):
            units.insert(int(len(units) * frac), (60, thunk))
            return units

        # E(0): proj(0); prefetch x(1) once pass-0's x consumption is done
        merge([insert_unit(proj_stream(0), 0.64, lambda: load_x(1, 0, EC))])
        # E(1): attn(0) x proj(1); prefetch x(2), then wo
        p1 = insert_unit(proj_stream(1), 0.4, lambda: load_x(2, 0, EC))
        insert_unit(p1, 0.8, lambda: nc.sync.dma_start(wo_v3[:], Wo_v3[:]))
        merge([attn_stream(0), p1])
        # E(2): attn(1) x proj(2) x outproj(0); prefetch x(3)
        merge([attn_stream(1),
               insert_unit(proj_stream(2), 0.5, lambda: load_x(3, 0, EC)),
               outproj_stream(0)])
        # E(3): attn(2) x proj(3) x outproj(1)
        merge([attn_stream(2), proj_stream(3), outproj_stream(1)])
        # E(4): attn(3) x outproj(2)
        merge([attn_stream(3), outproj_stream(2)])
        # E(5): outproj(3)
        merge([outproj_stream(3)])

        if debug:
            for h in range(NHL):
                nc.sync.dma_start(dbg_q[h][:], qkT[h][:])
                nc.sync.dma_start(dbg_y[h][:], yT[h][:])
            nc.sync.dma_start(dbg_k[:], qkT[NHL][:])
            nc.sync.dma_start(dbg_v[:], vA[:])

    nc.compile()
    return nc


def make_consts(S):
    rope_dim = D // 2
    j = np.arange(rope_dim, dtype=np.float64)
    thetas = 1.0 / ROPE_BASE ** (2.0 * j / rope_dim)
    positions = np.arange(S, dtype=np.float64)
    angles = positions[:, None] * thetas[None, :]
    sin = np.sin(np.concatenate([angles, angles], axis=1)).astype(np.float32)
    sinT = np.ascontiguousarray(sin.T)                       # [D, S]
    sgn = np.where(np.arange(D) < rope_dim, -1.0, 1.0).astype(np.float32)
    sinTs = np.ascontiguousarray(sinT * sgn[:, None])

    k_idx = np.arange(128)[:, None]
    q_idx = np.arange(128)[None, :]
    mask = (k_idx <= q_idx).astype(np.float32)               # lower-tri in [k,q]

    return {
        "sinT": sinT.astype(NPBF16),
        "sinTs": sinTs.astype(NPBF16),
        "mask_in": np.ascontiguousarray(mask).astype(NPBF16),
        "ones_col": np.ones((128, 1), NPBF16),
        "ones_row": np.ones((1, 128), NPBF16),
    }


def make_in_maps(x, Wq, Wk, Wv, Wo, S, E):
    consts = make_consts(S)
    in_maps = []
    for c in range(N_CORES):
        b, g = c // TP, c % TP
        m = dict(consts)
        m["xT"] = np.ascontiguousarray(x[b].T).astype(NPBF16)
        m["Wq"] = np.ascontiguousarray(
            Wq[:, NHL * D * g:NHL * D * (g + 1)]).astype(NPBF16)
        m["Wk"] = np.ascontiguousarray(Wk[:, D * g:D * (g + 1)]).astype(NPBF16)
        m["Wv"] = np.ascontiguousarray(Wv[:, D * g:D * (g + 1)]).astype(NPBF16)
        m["Wo"] = np.ascontiguousarray(
            Wo[NHL * D * g:NHL * D * (g + 1), :]).astype(NPBF16)
        in_maps.append(m)
    return in_maps


_CACHE = {}


def _compiled_full():
    if "nc" not in _CACHE:
        _CACHE["nc"] = build_program(S_FULL, E_FULL)
    return _CACHE["nc"]


def kernel(x, Wq, Wk, Wv, Wo, bo):
    nc = _compiled_full()
    in_maps = make_in_maps(x, Wq, Wk, Wv, Wo, S_FULL, E_FULL)
    res = run_bass_kernel_spmd(nc, in_maps, list(range(N_CORES)))
    out = np.zeros((BATCH, S_FULL, E_FULL), np.float32)
    for c in range(N_CORES):
        out[c // TP] += res.results[c]["out"].astype(np.float32)
    out += bo.astype(np.float32)[None, None, :]
    return out


# revision 3
# speedup vs baseline: 1.0051x; 1.0051x over previous
"""Trainium2 Bass kernel: causal multi-head group attention (GQA) with RoPE.

v2: bf16 datapath + software-pipelined phase interleaving + engine rebalance.

Full-input contract: kernel(**inputs) takes the unsharded inputs and returns
the full output. Internally shards across 8 NeuronCores:
  core c -> (batch b = c // 4, head-group g = c % 4)
Each core computes 4 q heads + their single kv group end-to-end. The host
unshard step sums the 4 row-parallel out-proj partials and adds the bias.

Design notes (cost-model driven):
 - All matmul operands bf16 (1 cycle/row at any output width); PSUM f32.
 - Emission interleaves proj(sp) / attention(sp-1) / out-proj(sp-2) so the
   tensor engine (the pacing engine, ~205us of work) never idles while the
   Act engine grinds through exp (the attention pacer).
 - Engine assignment: exp -> Act; rope / rs-accum / mask / reciprocal -> DVE
   (bf16 SBUF-only ops hit the 4x DVE mode); softmax y-scale and out-proj
   PSUM->SBUF copies -> Pool (otherwise idle); bias add -> host.
 - V is projected with x as the stationary operand, yielding [s, d] tiles
   directly (no PE transposes, single PSUM->SBUF copy per pass).
"""

import os
import sys
from contextlib import ExitStack, nullcontext
from math import sqrt

for _p in ("/opt/trn_rl_repo", "/root/.axon_site/_ro/trn_rl_repo"):
    if os.path.isdir(_p) and _p not in sys.path:
        sys.path.insert(0, _p)

import numpy as np
import concourse.bacc as bacc
import concourse.tile as tile
import concourse.mybir as mybir
from concourse.bass_utils import run_bass_kernel_spmd

F32 = mybir.dt.float32
BF16 = mybir.dt.bfloat16
EXP = mybir.ActivationFunctionType.Exp
NPBF16 = mybir.dt.np(BF16)

N_CORES = 8
TP = 4            # head-group parallel degree (within one batch element)
BATCH = 2
D = 128           # head dim
NHL = 4           # q heads per core
ROPE_BASE = 10000.0

S_FULL = 2048     # context length
E_FULL = 2048     # model dim


def build_program(S, E, QC=512, PW=512, n_cores=N_CORES, reps=1, debug=False):
    EC = E // 128     # contraction chunks over model dim
    NSP = S // PW     # passes over the sequence (also q-chunks: QC == PW)
    assert QC == PW == 512
    scale = 1.0 / sqrt(D)

    nc = bacc.Bacc("TRN2", target_bir_lowering=False, debug=False,
                   num_devices=n_cores)

    xT = nc.dram_tensor("xT", [E, S], BF16, kind="ExternalInput").ap()
    Wq = nc.dram_tensor("Wq", [E, NHL * D], BF16, kind="ExternalInput").ap()
    Wk = nc.dram_tensor("Wk", [E, D], BF16, kind="ExternalInput").ap()
    Wv = nc.dram_tensor("Wv", [E, D], BF16, kind="ExternalInput").ap()
    Wo = nc.dram_tensor("Wo", [NHL * D, E], BF16, kind="ExternalInput").ap()
    sinT = nc.dram_tensor("sinT", [D, S], BF16, kind="ExternalInput").ap()
    sinTs = nc.dram_tensor("sinTs", [D, S], BF16, kind="ExternalInput").ap()
    mask_in = nc.dram_tensor("mask_in", [128, 128], BF16, kind="ExternalInput").ap()
    ones_col = nc.dram_tensor("ones_col", [128, 1], BF16, kind="ExternalInput").ap()
    ones_row = nc.dram_tensor("ones_row", [1, 128], BF16, kind="ExternalInput").ap()
    out = nc.dram_tensor("out", [S, E], BF16, kind="ExternalOutput").ap()
    if debug:
        dbg_q = [nc.dram_tensor(f"dbg_q{h}", [128, S], BF16,
                                kind="ExternalOutput").ap() for h in range(NHL)]
        dbg_k = nc.dram_tensor("dbg_k", [128, S], BF16,
                               kind="ExternalOutput").ap()
        dbg_v = nc.dram_tensor("dbg_v", [128, S], BF16,
                               kind="ExternalOutput").ap()
        dbg_y = [nc.dram_tensor(f"dbg_y{h}", [128, S], BF16,
                                kind="ExternalOutput").ap() for h in range(NHL)]

    with tile.TileContext(nc) as tc, \
         (tc.For_i(0, reps, 1) if reps > 1 else nullcontext()), \
         ExitStack() as top:
        pers = top.enter_context(tc.tile_pool(name="pers", bufs=1))
        qkT = [pers.tile([128, S], BF16, tag=f"qkT{g}", name=f"qkT{g}")
               for g in range(NHL + 1)]      # 4 q heads + k, [d, s] layout
        vA = pers.tile([128, S], BF16, name="vA")      # [:,128k:128k+128]=[s,d]
        yT = [pers.tile([128, S], BF16, tag=f"yT{h}", name=f"yT{h}")
              for h in range(NHL)]
        wq = pers.tile([128, EC * NHL * D], BF16, tag="wq", name="wq")
        wk = pers.tile([128, EC * D], BF16, tag="wk", name="wk")
        wv = pers.tile([128, EC * D], BF16, tag="wv", name="wv")
        wo = pers.tile([128, NHL * E], BF16, tag="wo", name="wo")
        sin_sb = pers.tile([128, S], BF16, tag="sin", name="sin_sb")
        sins_sb = pers.tile([128, S], BF16, tag="sins", name="sins_sb")
        mask_sb = pers.tile([128, 128], BF16, tag="mask", name="mask_sb")
        onesc = pers.tile([128, 1], BF16, tag="onesc", name="onesc")
        onesr = pers.tile([1, 128], BF16, tag="onesr", name="onesr")

        xpool = top.enter_context(tc.tile_pool(name="xt", bufs=2))
        opool = top.enter_context(tc.tile_pool(name="osb", bufs=2))
        hot = top.enter_context(tc.tile_pool(name="hot", bufs=1))
        psum = top.enter_context(tc.tile_pool(name="psum", bufs=1, space="PSUM"))

        xT_v = xT.rearrange("(a p) m -> p a m", p=128)
        wq_v3 = wq.rearrange("p (a n) -> p a n", a=EC)
        Wq_v3 = Wq.rearrange("(a p) n -> p a n", p=128)
        wo_v3 = wo.rearrange("p (a n) -> p a n", a=NHL)
        Wo_v3 = Wo.rearrange("(a p) n -> p a n", p=128)

        xt = [None] * NSP  # per-pass x tiles

        def load_x(sp, e0, e1):
            """One DMA loading x chunks [e0, e1) for pass sp."""
            if xt[sp] is None:
                xt[sp] = xpool.tile([128, EC * PW], BF16, tag="xt",
                                    name=f"xt{sp}")
            v3 = xt[sp].rearrange("p (a m) -> p a m", a=EC)
            nc.sync.dma_start(v3[:, e0:e1, :],
                              xT_v[:, e0:e1, PW * sp:PW * (sp + 1)])

        def emit_xdma(sp):
            return [(60, lambda sp=sp: load_x(sp, 0, EC))]

        # ------------- stream builders (lists of (pe_ns, thunk)) ----------
        def proj_stream(sp):
            units = []
            lo, hi = PW * sp, PW * (sp + 1)

            def xsl(e):
                return xt[sp][:, PW * e:PW * (e + 1)]

            def qk_stat(g, e):
                if g < NHL:
                    return wq[:, NHL * D * e + D * g:NHL * D * e + D * (g + 1)]
                return wk[:, D * e:D * (e + 1)]

            def rope_of(g, ps, lo=lo, hi=hi, sp=sp):
                full = qkT[g]
                t = full[:, lo:hi]
                nc.vector.tensor_copy(t, ps[:])
                tmp = hot.tile([128, PW], BF16, tag="ropetmp", bufs=2,
                               name=f"rt{sp}_{g}")
                nc.sync.dma_start(tmp[0:64, :], full[64:128, lo:hi])
                nc.sync.dma_start(tmp[64:128, :], full[0:64, lo:hi])
                nc.vector.tensor_mul(tmp[:], tmp[:], sins_sb[:, lo:hi])
                nc.vector.tensor_mul(t, t, sin_sb[:, lo:hi])
                nc.vector.tensor_add(t, t, tmp[:])

            if sp == 0:
                # Pass 0 runs e-major across all 5 q/k groups (one PSUM bank
                # each, via distinct tags) so the PE consumes x chunks as the
                # startup DMAs deliver them instead of stalling group-major.
                tags = ["proj", "proj", "st", "st", "yps"]
                ps0 = [psum.tile([128, PW], F32, tag=tags[g], bufs=2,
                                 name=f"pj0_{g}") for g in range(NHL + 1)]

                def emm(e):
                    for g in range(NHL + 1):
                        nc.tensor.matmul(ps0[g][:], qk_stat(g, e), xsl(e),
                                         start=(e == 0), stop=(e == EC - 1))
                for e in range(EC):
                    units.append((1065, lambda e=e: emm(e)))
                for g in range(NHL + 1):
                    units.append((100, lambda g=g: rope_of(g, ps0[g])))
            else:
                # 4 q heads + k: groups of 16 accumulating matmuls -> copy
                for g in range(NHL + 1):
                    ps_box = []

                    def mm4(e0, g=g, ps_box=ps_box, sp=sp):
                        if not ps_box:
                            ps_box.append(psum.tile([128, PW], F32, tag="proj",
                                                    bufs=2, name=f"pj{sp}_{g}"))
                        ps = ps_box[0]
                        for e in range(e0, e0 + 4):
                            nc.tensor.matmul(ps[:], qk_stat(g, e), xsl(e),
                                             start=(e == 0), stop=(e == EC - 1))
                    for e0 in range(0, EC, 4):
                        units.append((854, lambda e0=e0, f=mm4: f(e0)))
                    units.append((100, lambda g=g, ps_box=ps_box:
                                  rope_of(g, ps_box[0])))

            # v: [s, d] tiles via x-stationary matmuls. One accumulation
            # group per 128-row s-chunk, each in its own PSUM tile: a matmul
            # with start=True zeroes its whole PSUM bank, so interleaved
            # groups must not share one.
            def vgroup(j):
                vt = psum.tile([128, D], F32, tag="proj", bufs=2,
                               name=f"vps{sp}_{j}")
                for e in range(EC):
                    nc.tensor.matmul(
                        vt[:],
                        xt[sp][:, PW * e + 128 * j:PW * e + 128 * (j + 1)],
                        wv[:, D * e:D * (e + 1)],
                        start=(e == 0), stop=(e == EC - 1))
                nc.vector.tensor_copy(
                    vA[:, lo + 128 * j:lo + 128 * (j + 1)], vt[:])
            for j in range(4):
                units.append((900, lambda j=j: vgroup(j)))
            return units

        def attn_stream(sp):
            units = []
            nki = 4 * (sp + 1)
            for h in range(NHL):
                yps_box, rs_box = [], []

                def tilework(ki, h=h, yps_box=yps_box, rs_box=rs_box, sp=sp,
                             nki=nki):
                    if not yps_box:
                        yps_box.append(psum.tile([128, QC], F32, tag="yps",
                                                 bufs=2, name=f"yps{h}_{sp}"))
                        rs_box.append(hot.tile([128, QC], BF16, tag="rs",
                                               bufs=3, name=f"rs{h}_{sp}"))
                    yps, rs = yps_box[0], rs_box[0]
                    off = 128 * ki - QC * sp
                    qlo = max(0, off)
                    st = psum.tile([128, QC], F32, tag="st", bufs=2,
                                   name=f"st{h}_{sp}_{ki}")
                    nc.tensor.matmul(
                        st[:, qlo:QC], qkT[NHL][:, 128 * ki:128 * (ki + 1)],
                        qkT[h][:, QC * sp + qlo:QC * (sp + 1)],
                        start=True, stop=True)
                    if ki == 0:
                        dst = rs
                    else:
                        dst = hot.tile([128, QC], BF16, tag="pt", bufs=6,
                                       name=f"pt{h}_{sp}_{ki}")
                    nc.scalar.activation(dst[:, qlo:QC], st[:, qlo:QC], EXP,
                                         scale=scale)
                    if off >= 0:
                        nc.vector.tensor_mul(dst[:, qlo:qlo + 128],
                                             dst[:, qlo:qlo + 128], mask_sb[:])
                    if ki != 0:
                        nc.vector.tensor_add(rs[:, qlo:QC], rs[:, qlo:QC],
                                             dst[:, qlo:QC])
                    nc.tensor.matmul(yps[:, qlo:QC],
                                     vA[:, 128 * ki:128 * (ki + 1)],
                                     dst[:, qlo:QC],
                                     start=(ki == 0), stop=(ki == nki - 1))
                for ki in range(nki):
                    units.append((430, lambda ki=ki, f=tilework: f(ki)))

                rinv_box = []

                def fin_a(h=h, rs_box=rs_box, rinv_box=rinv_box, sp=sp):
                    rsum = psum.tile([1, QC], F32, tag="aux", bufs=2,
                                     name=f"rsum{h}_{sp}")
                    nc.tensor.matmul(rsum[:], onesc[:], rs_box[0][:],
                                     start=True, stop=True)
                    rinv = hot.tile([1, QC], BF16, tag="rinv", bufs=2,
                                    name=f"rinv{h}_{sp}")
                    with nc.allow_low_precision(reason="bf16 softmax denom"):
                        nc.vector.reciprocal(rinv[:], rsum[:])
                    rinv_box.append(rinv)

                def fin_b(h=h, yps_box=yps_box, rinv_box=rinv_box, sp=sp):
                    # separate unit: the merger interleaves PE filler between
                    # fin_a and fin_b so the rb matmul doesn't stall the
                    # in-order PE queue waiting on the DVE reciprocal
                    rb = psum.tile([128, QC], F32, tag="aux", bufs=2,
                                   name=f"rb{h}_{sp}")
                    nc.tensor.matmul(rb[:], onesr[:], rinv_box[0][:],
                                     start=True, stop=True)
                    rb_sb = hot.tile([128, QC], F32, tag="rbs", bufs=2,
                                     name=f"rbs{h}_{sp}")
                    nc.scalar.copy(rb_sb[:], rb[:])
                    with nc.allow_low_precision(reason="bf16 y"):
                        nc.vector.tensor_mul(yT[h][:, QC * sp:QC * (sp + 1)],
                                             yps_box[0][:], rb_sb[:])
                units.append((215, fin_a))
                units.append((215, fin_b))
            return units

        def outproj_stream(sp):
            units = []
            for si in range(4 * sp, 4 * (sp + 1)):
                osb_box = []

                def njwork(nj, si=si, osb_box=osb_box, sp=sp):
                    if not osb_box:
                        osb_box.append(opool.tile([128, E], BF16, tag="osb",
                                                  name=f"osb{si}"))
                    ops = psum.tile([128, 512], F32, tag="aux", bufs=2,
                                    name=f"ops{si}_{nj}")
                    for h in range(NHL):
                        nc.tensor.matmul(
                            ops[:], yT[h][:, 128 * si:128 * (si + 1)],
                            wo[:, E * h + 512 * nj:E * h + 512 * (nj + 1)],
                            start=(h == 0), stop=(h == NHL - 1))
                    dst = osb_box[0][:, 512 * nj:512 * (nj + 1)]
                    # Pool can't read PSUM; Act has slack in early passes
                    # (light attention), DVE in late ones (no projections);
                    # final pass alternates so the tail drains on two engines
                    if sp < 2 or (sp == NSP - 1 and nj % 2 == 0):
                        nc.scalar.copy(dst, ops[:])
                    else:
                        nc.vector.tensor_copy(dst, ops[:])
                for nj in range(E // 512):
                    units.append((852, lambda nj=nj, f=njwork: f(nj)))

                def outdma(si=si, osb_box=osb_box, sp=sp):
                    rows = out[128 * si:128 * (si + 1), :]
                    if sp == NSP - 1 and si == 4 * (sp + 1) - 1:
                        # final chunk: 4 smaller DMAs so the tail drains early
                        for nj in range(4):
                            nc.sync.dma_start(
                                rows[:, 512 * nj:512 * (nj + 1)],
                                osb_box[0][:, 512 * nj:512 * (nj + 1)])
                    else:
                        nc.sync.dma_start(rows, osb_box[0][:])
                units.append((60, outdma))
            return units

        def merge(streams):
            streams = [s for s in streams if s]
            totals = [sum(w for w, _ in s) for s in streams]
            prog = [0.0] * len(streams)
            idx = [0] * len(streams)
            while True:
                cands = [i for i in range(len(streams))
                         if idx[i] < len(streams[i])]
                if not cands:
                    break
                i = min(cands, key=lambda i: prog[i] / totals[i])
                w, t = streams[i][idx[i]]
                idx[i] += 1
                prog[i] += w
                t()

        # -------------------- preload + pipeline schedule -----------------
        # Startup order tuned for serialized HWDGE descriptor generation:
        # block-interleave wq / wk / x0 so pass-0's e-major projection can
        # consume chunks as they land; tables/constants follow (their
        # consumers tolerate the latency).
        wk_v3 = wk.rearrange("p (a n) -> p a n", a=EC)
        Wk_v3 = Wk.rearrange("(a p) n -> p a n", p=128)
        for b0, b1 in ((0, 1), (1, 2), (2, 4), (4, 8), (8, 12), (12, 16)):
            nc.sync.dma_start(wq_v3[:, b0:b1, :], Wq_v3[:, b0:b1, :])
            nc.sync.dma_start(wk_v3[:, b0:b1, :], Wk_v3[:, b0:b1, :])
            load_x(0, b0, b1)
        nc.sync.dma_start(wv.rearrange("p (a n) -> p a n", a=EC),
                          Wv.rearrange("(a p) n -> p a n", p=128))
        nc.sync.dma_start(sin_sb[:], sinT[:])
        nc.sync.dma_start(sins_sb[:], sinTs[:])
        nc.sync.dma_start(mask_sb[:], mask_in[:])
        nc.sync.dma_start(onesc[:], ones_col[:])
        nc.sync.dma_start(onesr[:], ones_row[:])

        def insert_unit(units, frac, thunk):
            units.insert(int(len(units) * frac), (60, thunk))
            return units

        # E(0): proj(0); prefetch x(1) once pass-0's x consumption is done
        merge([insert_unit(proj_stream(0), 0.64, lambda: load_x(1, 0, EC))])
        # E(1): attn(0) x proj(1); prefetch x(2), then wo
        p1 = insert_unit(proj_stream(1), 0.4, lambda: load_x(2, 0, EC))
        insert_unit(p1, 0.8, lambda: nc.sync.dma_start(wo_v3[:], Wo_v3[:]))
        merge([attn_stream(0), p1])
        # E(2): attn(1) x proj(2) x outproj(0); prefetch x(3)
        merge([attn_stream(1),
               insert_unit(proj_stream(2), 0.5, lambda: load_x(3, 0, EC)),
               outproj_stream(0)])
        # E(3): attn(2) x proj(3) x outproj(1)
        merge([attn_stream(2), proj_stream(3), outproj_stream(1)])
        # E(4): attn(3) x outproj(2)
        merge([attn_stream(3), outproj_stream(2)])
        # E(5): outproj(3)
        merge([outproj_stream(3)])

        if debug:
            for h in range(NHL):
                nc.sync.dma_start(dbg_q[h][:], qkT[h][:])
                nc.sync.dma_start(dbg_y[h][:], yT[h][:])
            nc.sync.dma_start(dbg_k[:], qkT[NHL][:])
            nc.sync.dma_start(dbg_v[:], vA[:])

    nc.compile()
    return nc


def make_consts(S):
    rope_dim = D // 2
    j = np.arange(rope_dim, dtype=np.float64)
    thetas = 1.0 / ROPE_BASE ** (2.0 * j / rope_dim)
    positions = np.arange(S, dtype=np.float64)
    angles = positions[:, None] * thetas[None, :]
    sin = np.sin(np.concatenate([angles, angles], axis=1)).astype(np.float32)
    sinT = np.ascontiguousarray(sin.T)                       # [D, S]
    sgn = np.where(np.arange(D) < rope_dim, -1.0, 1.0).astype(np.float32)
    sinTs = np.ascontiguousarray(sinT * sgn[:, None])

    k_idx = np.arange(128)[:, None]
    q_idx = np.arange(128)[None, :]
    mask = (k_idx <= q_idx).astype(np.float32)               # lower-tri in [k,q]

    return {
        "sinT": sinT.astype(NPBF16),
        "sinTs": sinTs.astype(NPBF16),
        "mask_in": np.ascontiguousarray(mask).astype(NPBF16),
        "ones_col": np.ones((128, 1), NPBF16),
        "ones_row": np.ones((1, 128), NPBF16),
    }


def make_in_maps(x, Wq, Wk, Wv, Wo, S, E):
    consts = make_consts(S)
    in_maps = []
    for c in range(N_CORES):
        b, g = c // TP, c % TP
        m = dict(consts)
        m["xT"] = np.ascontiguousarray(x[b].T).astype(NPBF16)
        m["Wq"] = np.ascontiguousarray(
            Wq[:, NHL * D * g:NHL * D * (g + 1)]).astype(NPBF16)
        m["Wk"] = np.ascontiguousarray(Wk[:, D * g:D * (g + 1)]).astype(NPBF16)
        m["Wv"] = np.ascontiguousarray(Wv[:, D * g:D * (g + 1)]).astype(NPBF16)
        m["Wo"] = np.ascontiguousarray(
            Wo[NHL * D * g:NHL * D * (g + 1), :]).astype(NPBF16)
        in_maps.append(m)
    return in_maps


_CACHE = {}


def _compiled_full():
    if "nc" not in _CACHE:
        _CACHE["nc"] = build_program(S_FULL, E_FULL)
    return _CACHE["nc"]


def kernel(x, Wq, Wk, Wv, Wo, bo):
    nc = _compiled_full()
    in_maps = make_in_maps(x, Wq, Wk, Wv, Wo, S_FULL, E_FULL)
    res = run_bass_kernel_spmd(nc, in_maps, list(range(N_CORES)))
    out = np.zeros((BATCH, S_FULL, E_FULL), np.float32)
    for c in range(N_CORES):
        out[c // TP] += res.results[c]["out"].astype(np.float32)
    out += bo.astype(np.float32)[None, None, :]
    return out
